# revision 5
# baseline (speedup 1.0000x reference)
"""GCNConv kernel for 8 Trainium2 NeuronCores (Bass/Tile).

Computes out = segment_sum(edge_val * (x @ W)[edge_col], edge_row) + b
as out = (A @ x) @ W + b  (associativity), with:
  - nodes (rows of output) sharded across 8 cores (12500 each)
  - edges partitioned by destination row -> per-core, per-128-row-tile
  - per 128-edge block: gather x[col] rows (fp16, 512B) via dma_gather,
    build a one-hot selection matrix S[e, dloc[e]] = val[e], and
    accumulate z[128 nodes, 256] += S.T @ X_block on the PE in PSUM.
  - S is built on the DVE as uint16 integer ops on the f16 BIT PATTERNS:
    T[e, j] = (j == dloc[e]) * f16bits(val[e]), bitcast to f16. Exact, and
    the 16-bit dtype unlocks the DVE 2x/4x acceleration modes. A fraction
    of blocks goes to the scalar engine (Abs+Relu) for balance.
  - epilogue per tile: transpose z, project by W (fp16), add bias, store.

x is split into 4 banks of 25000 rows because dma_gather indices are int16.
"""
import os
from contextlib import ExitStack

import numpy as np

import concourse.bass as bass
import concourse.tile as tile
from concourse import bacc, mybir
from concourse.bass_utils import run_bass_kernel_spmd

P = 128
D = 256
N_NODES = 100000
N_EDGES = 3200000
NC = 8
SH = N_NODES // NC          # 12500 rows per core
NT = (SH + P - 1) // P      # 98 tiles per core
NBANK = 4
BS = N_NODES // NBANK       # 25000 rows per bank (fits int16 index)

F16 = mybir.dt.float16
F32 = mybir.dt.float32
I16 = mybir.dt.int16
U16 = mybir.dt.uint16

# block j (global index) goes to the scalar engine iff j % SCALAR_MOD == 3
SCALAR_MOD = 7
# max blocks per dma_gather call (896 idxs <= 992 keeps single_packet legal)
GB = 7

_last_results = None        # BassKernelResults of the most recent run


def _is_scalar_block(j):
    return j % SCALAR_MOD == 3


def _build_structure(edge_row, edge_col, edge_val):
    """Sort/pad edges into per-core 128-edge blocks grouped by
    (dest tile, source bank).  Block structure (nb_tk) is shared across
    cores (padded to the max) so one SPMD program fits all cores.

    Returns (nb_tk [NT,NBANK] int, per-core dict arrays).
    """
    E = edge_row.shape[0]
    core = edge_row // SH
    r_loc = edge_row - core * SH
    t = r_loc // P
    dloc = (r_loc % P).astype(np.int64)
    bank = edge_col // BS
    bidx = (edge_col % BS).astype(np.int16)

    gid = (core.astype(np.int64) * NT + t) * NBANK + bank
    order = np.argsort(gid, kind="stable")
    gid_s = gid[order]

    cnt = np.bincount(gid, minlength=NC * NT * NBANK).reshape(NC, NT, NBANK)
    nb_tk = (cnt.max(axis=0) + P - 1) // P          # [NT, NBANK] blocks
    nb_tk = np.maximum(nb_tk, 1)                     # keep structure non-empty
    NB_t = nb_tk.sum(axis=1)                         # [NT]
    NBLK = int(NB_t.sum())
    pad_len = NBLK * P

    # slot offset of group (t,k) within a core's padded edge list
    off_tk = np.zeros((NT, NBANK), np.int64)
    flat_off = np.cumsum(nb_tk.ravel() * P)
    off_tk.ravel()[1:] = flat_off[:-1]

    # position of each edge within its (c,t,k) group
    grp_start = np.zeros(E, np.int64)
    newgrp = np.ones(E, bool)
    newgrp[1:] = gid_s[1:] != gid_s[:-1]
    starts = np.where(newgrp)[0]
    grp_start[starts] = starts
    grp_start = np.maximum.accumulate(grp_start)
    pos_in_grp = np.arange(E) - grp_start

    tk_of_edge = gid_s % (NT * NBANK)
    core_of_edge = gid_s // (NT * NBANK)
    dest = off_tk.ravel()[tk_of_edge] + pos_in_grp

    # scalar-block bookkeeping (shared across cores)
    blk_scalar = np.array([_is_scalar_block(j) for j in range(NBLK)])
    s_ord = np.cumsum(blk_scalar) - blk_scalar      # ordinal among scalar blks
    NSBLK = int(blk_scalar.sum())

    cores = []
    ev16 = edge_val.astype(np.float16)
    vbits = ev16.view(np.uint16)
    for c in range(NC):
        m = core_of_edge == c
        e_ids = order[m]
        d = dest[m]
        idx_arr = np.zeros(pad_len, np.int16)
        dloc_arr = np.zeros(pad_len, np.int64)
        vb_arr = np.zeros(pad_len, np.uint16)
        vf_arr = np.zeros(pad_len, np.float32)
        idx_arr[d] = bidx[e_ids]
        dloc_arr[d] = dloc[e_ids]
        vb_arr[d] = vbits[e_ids]
        vf_arr[d] = edge_val[e_ids].astype(np.float32)

        # packed gather indices: [128, 8*NBLK] int16 (16-wrap, replicated x8)
        idxp = np.tile(np.ascontiguousarray(idx_arr.reshape(-1, 16).T), (8, 1))

        # DVE path: per block [dloc, float(f16bits(val))] -> [128, 2*NBLK] f32
        # (bass requires f32 scalars; ints <= 65535 are exact in f32)
        dl = np.ascontiguousarray(dloc_arr.reshape(NBLK, P).T)   # [128, NBLK]
        vb = np.ascontiguousarray(vb_arr.reshape(NBLK, P).T)
        dvu = np.empty((P, 2 * NBLK), np.float32)
        dvu[:, 0::2] = dl.astype(np.float32)
        dvu[:, 1::2] = vb.astype(np.float32)

        # scalar path: per scalar-block [-dloc, -val, +val] f32
        dvs = np.zeros((P, max(3 * NSBLK, 3)), np.float32)
        if NSBLK:
            dlf = dl.astype(np.float32)
            vf = np.ascontiguousarray(vf_arr.reshape(NBLK, P).T)
            sb = np.where(blk_scalar)[0]
            dvs[:, 0::3] = -dlf[:, sb]
            dvs[:, 1::3] = -vf[:, sb]
            dvs[:, 2::3] = vf[:, sb]
        cores.append(dict(idxp=idxp, dvu=dvu, dvs=dvs))

    return nb_tk, (blk_scalar, s_ord, NSBLK), cores


def _build_program(nb_tk, scal_info):
    """Build the SPMD Bass program for the given block structure."""
    nb_tk = np.asarray(nb_tk)
    blk_scalar, s_ord, NSBLK = scal_info
    NB_t = nb_tk.sum(axis=1)
    NBLK = int(NB_t.sum())
    nt = nb_tk.shape[0]
    out_rows = nt * P

    nc = bacc.Bacc("TRN2", target_bir_lowering=False, debug=False,
                   num_devices=NC, num_swdge_queues=4)
    xb_aps = [nc.dram_tensor(f"xb{k}", [BS, D], F16,
                             kind="ExternalInput").ap() for k in range(NBANK)]
    idxp_ap = nc.dram_tensor("idxp", [P, 8 * NBLK], I16,
                             kind="ExternalInput").ap()
    dvu_ap = nc.dram_tensor("dvu", [P, 2 * NBLK], F32,
                            kind="ExternalInput").ap()
    dvs_ap = nc.dram_tensor("dvs", [P, max(3 * NSBLK, 3)], F32,
                            kind="ExternalInput").ap()
    w_ap = nc.dram_tensor("w", [D, D], F16, kind="ExternalInput").ap()
    bias_ap = nc.dram_tensor("bias", [P, D], F32, kind="ExternalInput").ap()
    iota_ap = nc.dram_tensor("iota", [P, P], F16, kind="ExternalInput").ap()
    iotau_ap = nc.dram_tensor("iotau", [P, P], U16, kind="ExternalInput").ap()
    ident_ap = nc.dram_tensor("ident", [P, P], F16, kind="ExternalInput").ap()
    out_ap = nc.dram_tensor("out", [out_rows, D], F32,
                            kind="ExternalOutput").ap()

    nb_max = int(NB_t.max())

    with tile.TileContext(nc) as tc:
        with ExitStack() as ctx:
            const = ctx.enter_context(tc.tile_pool(name="const", bufs=1))
            idxpool = ctx.enter_context(tc.tile_pool(name="idxp", bufs=6))
            dvupool = ctx.enter_context(tc.tile_pool(name="dvup", bufs=6))
            dvspool = ctx.enter_context(tc.tile_pool(name="dvsp", bufs=6))
            xgpool = ctx.enter_context(tc.tile_pool(name="xgp", bufs=6))
            spool = ctx.enter_context(tc.tile_pool(name="sp", bufs=10))
            epool = ctx.enter_context(tc.tile_pool(name="ep", bufs=3))
            apool = ctx.enter_context(tc.tile_pool(name="ap", bufs=4))
            zpsum = ctx.enter_context(
                tc.tile_pool(name="zps", bufs=3, space="PSUM"))
            tpsum = ctx.enter_context(
                tc.tile_pool(name="tps", bufs=2, space="PSUM"))
            opsum = ctx.enter_context(
                tc.tile_pool(name="ops", bufs=2, space="PSUM"))

            iota_t = const.tile([P, P], F16, tag="iota")
            nc.sync.dma_start(iota_t[:], iota_ap[:])
            iotau_t = const.tile([P, P], U16, tag="iotau")
            nc.sync.dma_start(iotau_t[:], iotau_ap[:])
            ident_t = const.tile([P, P], F16, tag="ident")
            nc.sync.dma_start(ident_t[:], ident_ap[:])
            w_t = const.tile([P, 2, D], F16, tag="w")
            nc.sync.dma_start(w_t[:], w_ap[:].rearrange("(c k) d -> k c d",
                                                        k=P))
            bias_t = const.tile([P, D], F32, tag="bias")
            nc.sync.dma_start(bias_t[:], bias_ap[:])

            TGL = 4  # tiles per idx/dv load group (prefetch + fewer sync ops)
            grp_nb_max = max(int(NB_t[g:g + TGL].sum())
                             for g in range(0, nt, TGL))
            # scalar-block prefix offsets per group
            blk_of_t = np.concatenate([[0], np.cumsum(NB_t)]).astype(int)
            ns_before = np.concatenate([[0], np.cumsum(blk_scalar)]).astype(int)
            grp_ns_max = max(
                int(ns_before[blk_of_t[min(g + TGL, nt)]]
                    - ns_before[blk_of_t[g]]) for g in range(0, nt, TGL))
            grp_ns_max = max(grp_ns_max, 1)

            bo = 0  # global block offset
            idx_t = dvu_t = dvs_t = None
            gbo = 0   # block offset of current group start
            gso = 0   # scalar-block ordinal at current group start
            qc = 0    # rotating gather queue counter
            for t in range(nt):
                nb = int(NB_t[t])
                if t % TGL == 0:
                    gnb = int(NB_t[t:t + TGL].sum())
                    gbo = bo
                    idx_t = idxpool.tile([P, 8 * grp_nb_max], I16, tag="idx")
                    nc.sync.dma_start(idx_t[:, :8 * gnb],
                                      idxp_ap[:, 8 * bo:8 * (bo + gnb)])
                    dvu_t = dvupool.tile([P, 2 * grp_nb_max], F32, tag="dvu")
                    nc.sync.dma_start(dvu_t[:, :2 * gnb],
                                      dvu_ap[:, 2 * bo:2 * (bo + gnb)])
                    gso = int(ns_before[bo])
                    gns = int(ns_before[bo + gnb]) - gso
                    dvs_t = dvspool.tile([P, 3 * grp_ns_max], F32, tag="dvs")
                    if gns:
                        nc.sync.dma_start(dvs_t[:, :3 * gns],
                                          dvs_ap[:, 3 * gso:3 * (gso + gns)])
                lo = bo - gbo   # tile's block offset within the group tiles
                xg = xgpool.tile([P, nb_max, D], F16, tag="xg")
                ok = 0
                for k in range(NBANK):
                    nbk = int(nb_tk[t, k])
                    if nbk == 0:
                        continue
                    # split into <=GB-block calls so each is one SDMA packet
                    # (balanced: 8 blocks -> 4+4, not 7+1)
                    nch = -(-nbk // GB)
                    csz = -(-nbk // nch)
                    j0 = 0
                    while j0 < nbk:
                        cb = min(csz, nbk - j0)
                        n = cb * P
                        a = ok + j0
                        nc.gpsimd.dma_gather(
                            out_ap=xg[:, a:a + cb, :],
                            in_ap=xb_aps[k][:],
                            idxs_ap=idx_t[:, 8 * (lo + a):8 * (lo + a + cb)],
                            num_idxs=n,
                            num_idxs_reg=n,
                            elem_size=D,
                            single_packet=True,
                            queue_num=qc % 4,
                        )
                        qc += 1
                        j0 += cb
                    ok += nbk

                z_ps = zpsum.tile([P, D], F32, tag="zps")
                for j in range(nb):
                    gj = bo + j
                    s_t = spool.tile([P, P], F16, tag="s")
                    if _is_scalar_block(gj):
                        # scalar engine: S = relu(val - val*|iota - dloc|)
                        c = 3 * (int(s_ord[gj]) - gso)
                        a_t = apool.tile([P, P], F16, tag="at")
                        nc.scalar.activation(
                            a_t[:], iota_t[:],
                            mybir.ActivationFunctionType.Abs,
                            bias=dvs_t[:, c:c + 1])
                        nc.scalar.activation(
                            s_t[:], a_t[:],
                            mybir.ActivationFunctionType.Relu,
                            scale=dvs_t[:, c + 1:c + 2],
                            bias=dvs_t[:, c + 2:c + 3])
                    else:
                        # DVE: integer ops on the f16 bit patterns (exact)
                        c = 2 * (lo + j)
                        nc.vector.tensor_scalar(
                            out=s_t[:].bitcast(U16), in0=iotau_t[:],
                            scalar1=dvu_t[:, c:c + 1],
                            scalar2=dvu_t[:, c + 1:c + 2],
                            op0=mybir.AluOpType.is_equal,
                            op1=mybir.AluOpType.mult,
                        )
                    nc.tensor.matmul(out=z_ps[:], lhsT=s_t[:],
                                     rhs=xg[:, j, :],
                                     start=(j == 0), stop=(j == nb - 1))

                z_sb = epool.tile([P, D], F16, tag="zsb")
                nc.scalar.copy(z_sb[:], z_ps[:])
                o_ps = opsum.tile([P, D], F32, tag="ops")
                for ch in range(2):
                    zt_ps = tpsum.tile([P, P], F16, tag="ztps")
                    nc.tensor.transpose(zt_ps[:],
                                        z_sb[:, ch * P:(ch + 1) * P],
                                        ident_t[:])
                    zt_sb = epool.tile([P, P], F16, tag="ztsb")
                    nc.scalar.copy(zt_sb[:], zt_ps[:])
                    nc.tensor.matmul(out=o_ps[:], lhsT=zt_sb[:],
                                     rhs=w_t[:, ch, :],
                                     start=(ch == 0), stop=(ch == 1))
                o_sb = epool.tile([P, D], F32, tag="osb")
                nc.vector.tensor_add(o_sb[:], o_ps[:], bias_t[:])
                nc.sync.dma_start(out_ap[t * P:(t + 1) * P, :], o_sb[:])
                bo += nb
    nc.compile()
    return nc


def kernel(x, edge_row, edge_col, edge_val, weight, b):
    global _last_results
    assert x.shape == (N_NODES, D)

    nb_tk, scal_info, cores = _build_structure(
        np.asarray(edge_row), np.asarray(edge_col), np.asarray(edge_val))
    nc = _build_program(nb_tk, scal_info)

    x16 = np.asarray(x, np.float32).astype(np.float16)
    banks = [np.ascontiguousarray(x16[k * BS:(k + 1) * BS])
             for k in range(NBANK)]
    w16 = np.asarray(weight, np.float32).astype(np.float16)
    bias = np.broadcast_to(
        np.asarray(b, np.float32)[None, :], (P, D)).copy()
    iota = np.tile(np.arange(P, dtype=np.float16)[None, :], (P, 1))
    iotau = np.tile(np.arange(P, dtype=np.uint16)[None, :], (P, 1))
    ident = np.eye(P, dtype=np.float16)

    in_maps = []
    for c in range(NC):
        m = {f"xb{k}": banks[k] for k in range(NBANK)}
        m.update(idxp=cores[c]["idxp"], dvu=cores[c]["dvu"],
                 dvs=cores[c]["dvs"], w=w16, bias=bias, iota=iota,
                 iotau=iotau, ident=ident)
        in_maps.append(m)

    trace = bool(os.environ.get("KERNEL_TRACE"))
    res = run_bass_kernel_spmd(nc, in_maps, list(range(NC)), trace=trace)
    _last_results = res

    out = np.concatenate([res.results[c]["out"][:SH] for c in range(NC)],
                         axis=0)
    return out.astype(np.float32)


# revision 13
# speedup vs baseline: 1.3932x; 1.3932x over previous
"""GCNConv kernel for 8 Trainium2 NeuronCores (Bass/Tile).

Computes out = segment_sum(edge_val * (x @ W)[edge_col], edge_row) + b
as out = (A @ x) @ W + b  (associativity), with:
  - nodes (rows of output) sharded across 8 cores (12500 each)
  - edges partitioned by destination row -> per-core, per-128-row-tile
  - per 128-edge block: gather x[col] rows (fp16, 512B) via dma_gather,
    build a one-hot selection matrix S[e, dloc[e]] = val[e], and
    accumulate z[128 nodes, 256] += S.T @ X_block on the PE in PSUM.
  - S is built on the DVE as uint16 integer ops on the f16 BIT PATTERNS:
    T[e, j] = (j == dloc[e]) * f16bits(val[e]), bitcast to f16. Exact, and
    the 16-bit dtype unlocks the DVE 2x/4x acceleration modes. A fraction
    of blocks goes to the scalar engine (Abs+Relu) for balance.
  - epilogue per tile: transpose z, project by W (fp16), add bias, store.

x is split into 4 banks of 25000 rows because dma_gather indices are int16.
"""
import os
from contextlib import ExitStack

import numpy as np

import concourse.bass as bass
import concourse.tile as tile
from concourse import bacc, mybir
from concourse.bass_utils import run_bass_kernel_spmd

P = 128
D = 256
N_NODES = 100000
N_EDGES = 3200000
NC = 8
SH = N_NODES // NC          # 12500 rows per core
NT = (SH + P - 1) // P      # 98 tiles per core
NBANK = 4
BS = N_NODES // NBANK       # 25000 rows per bank (fits int16 index)

F16 = mybir.dt.float16
F32 = mybir.dt.float32
I16 = mybir.dt.int16
U16 = mybir.dt.uint16

XG_BUFS = 6                 # gathered-x tile pool depth

# S-build engine assignment by global block index: ~22% of blocks go to
# the scalar engine (Abs+Relu), the rest to the DVE.
_last_results = None        # BassKernelResults of the most recent run


def _is_scalar_block(j):
    return j % 9 in (3, 7)


def _is_gpsimd_block(j):
    return False


def _build_structure(edge_row, edge_col, edge_val):
    """Sort/pad edges into per-core 128-edge blocks grouped by
    (dest tile, source bank).  Block structure (nb_tk) is shared across
    cores (padded to the max) so one SPMD program fits all cores.

    Returns (nb_tk [NT,NBANK] int, per-core dict arrays).
    """
    E = edge_row.shape[0]
    core = edge_row // SH
    r_loc = edge_row - core * SH
    t = r_loc // P
    dloc = (r_loc % P).astype(np.int64)
    bank = edge_col // BS
    bidx = (edge_col % BS).astype(np.int16)

    gid = (core.astype(np.int64) * NT + t) * NBANK + bank
    order = np.argsort(gid, kind="stable")
    gid_s = gid[order]

    cnt = np.bincount(gid, minlength=NC * NT * NBANK).reshape(NC, NT, NBANK)
    nb_tk = (cnt.max(axis=0) + P - 1) // P          # [NT, NBANK] blocks
    nb_tk = np.maximum(nb_tk, 1)                     # keep structure non-empty
    NB_t = nb_tk.sum(axis=1)                         # [NT]
    NBLK = int(NB_t.sum())
    pad_len = NBLK * P

    # slot offset of group (t,k) within a core's padded edge list
    off_tk = np.zeros((NT, NBANK), np.int64)
    flat_off = np.cumsum(nb_tk.ravel() * P)
    off_tk.ravel()[1:] = flat_off[:-1]

    # position of each edge within its (c,t,k) group
    grp_start = np.zeros(E, np.int64)
    newgrp = np.ones(E, bool)
    newgrp[1:] = gid_s[1:] != gid_s[:-1]
    starts = np.where(newgrp)[0]
    grp_start[starts] = starts
    grp_start = np.maximum.accumulate(grp_start)
    pos_in_grp = np.arange(E) - grp_start

    tk_of_edge = gid_s % (NT * NBANK)
    core_of_edge = gid_s // (NT * NBANK)
    dest = off_tk.ravel()[tk_of_edge] + pos_in_grp

    # scalar-block bookkeeping (shared across cores)
    blk_scalar = np.array([_is_scalar_block(j) for j in range(NBLK)])
    s_ord = np.cumsum(blk_scalar) - blk_scalar      # ordinal among scalar blks
    NSBLK = int(blk_scalar.sum())

    cores = []
    ev16 = edge_val.astype(np.float16)
    vbits = ev16.view(np.uint16)
    for c in range(NC):
        m = core_of_edge == c
        e_ids = order[m]
        d = dest[m]
        idx_arr = np.zeros(pad_len, np.int16)
        dloc_arr = np.zeros(pad_len, np.int64)
        vb_arr = np.zeros(pad_len, np.uint16)
        vf_arr = np.zeros(pad_len, np.float32)
        idx_arr[d] = bidx[e_ids]
        dloc_arr[d] = dloc[e_ids]
        vb_arr[d] = vbits[e_ids]
        vf_arr[d] = edge_val[e_ids].astype(np.float32)

        # packed gather indices: [128, 8*NBLK] int16 (16-wrap, replicated x8)
        idxp = np.tile(np.ascontiguousarray(idx_arr.reshape(-1, 16).T), (8, 1))

        # DVE path: per block [dloc, float(f16bits(val))] -> [128, 2*NBLK] f32
        # (bass requires f32 scalars; ints <= 65535 are exact in f32)
        dl = np.ascontiguousarray(dloc_arr.reshape(NBLK, P).T)   # [128, NBLK]
        vb = np.ascontiguousarray(vb_arr.reshape(NBLK, P).T)
        dvu = np.empty((P, 2 * NBLK), np.float32)
        dvu[:, 0::2] = dl.astype(np.float32)
        dvu[:, 1::2] = vb.astype(np.float32)

        # scalar path: per scalar-block [-dloc, -val, +val] f32
        dvs = np.zeros((P, max(3 * NSBLK, 3)), np.float32)
        if NSBLK:
            dlf = dl.astype(np.float32)
            vf = np.ascontiguousarray(vf_arr.reshape(NBLK, P).T)
            sb = np.where(blk_scalar)[0]
            dvs[:, 0::3] = -dlf[:, sb]
            dvs[:, 1::3] = -vf[:, sb]
            dvs[:, 2::3] = vf[:, sb]
        cores.append(dict(idxp=idxp, dvu=dvu, dvs=dvs))

    return nb_tk, (blk_scalar, s_ord, NSBLK), cores


def _build_program(nb_tk, scal_info):
    """Build the SPMD Bass program for the given block structure."""
    nb_tk = np.asarray(nb_tk)
    blk_scalar, s_ord, NSBLK = scal_info
    NB_t = nb_tk.sum(axis=1)
    NBLK = int(NB_t.sum())
    nt = nb_tk.shape[0]
    out_rows = nt * P

    nc = bacc.Bacc("TRN2", target_bir_lowering=False, debug=False,
                   num_devices=NC, num_swdge_queues=4)
    xb_aps = [nc.dram_tensor(f"xb{k}", [BS, D], F16,
                             kind="ExternalInput").ap() for k in range(NBANK)]
    idxp_ap = nc.dram_tensor("idxp", [P, 8 * NBLK], I16,
                             kind="ExternalInput").ap()
    dvu_ap = nc.dram_tensor("dvu", [P, 2 * NBLK], F32,
                            kind="ExternalInput").ap()
    dvs_ap = nc.dram_tensor("dvs", [P, max(3 * NSBLK, 3)], F32,
                            kind="ExternalInput").ap()
    w_ap = nc.dram_tensor("w", [D, D], F16, kind="ExternalInput").ap()
    bias_ap = nc.dram_tensor("bias", [P, D], F32, kind="ExternalInput").ap()
    iota_ap = nc.dram_tensor("iota", [P, P], F16, kind="ExternalInput").ap()
    iotau_ap = nc.dram_tensor("iotau", [P, P], U16, kind="ExternalInput").ap()
    ident_ap = nc.dram_tensor("ident", [P, P], F16, kind="ExternalInput").ap()
    out_ap = nc.dram_tensor("out", [out_rows, D], F32,
                            kind="ExternalOutput").ap()

    nb_max = int(NB_t.max())

    with tile.TileContext(nc) as tc:
        with ExitStack() as ctx:
            const = ctx.enter_context(tc.tile_pool(name="const", bufs=1))
            idxpool = ctx.enter_context(tc.tile_pool(name="idxp", bufs=6))
            dvupool = ctx.enter_context(tc.tile_pool(name="dvup", bufs=6))
            dvspool = ctx.enter_context(tc.tile_pool(name="dvsp", bufs=6))
            xgpool = ctx.enter_context(tc.tile_pool(name="xgp", bufs=XG_BUFS))
            spool = ctx.enter_context(tc.tile_pool(name="sp", bufs=10))
            epool = ctx.enter_context(tc.tile_pool(name="ep", bufs=3))
            apool = ctx.enter_context(tc.tile_pool(name="ap", bufs=4))
            zpsum = ctx.enter_context(
                tc.tile_pool(name="zps", bufs=3, space="PSUM"))
            tpsum = ctx.enter_context(
                tc.tile_pool(name="tps", bufs=2, space="PSUM"))
            opsum = ctx.enter_context(
                tc.tile_pool(name="ops", bufs=2, space="PSUM"))

            iota_t = const.tile([P, P], F16, tag="iota")
            nc.sync.dma_start(iota_t[:], iota_ap[:])
            iotau_t = const.tile([P, P], U16, tag="iotau")
            nc.sync.dma_start(iotau_t[:], iotau_ap[:])
            ident_t = const.tile([P, P], F16, tag="ident")
            nc.sync.dma_start(ident_t[:], ident_ap[:])
            w_t = const.tile([P, 2, D], F16, tag="w")
            nc.sync.dma_start(w_t[:], w_ap[:].rearrange("(c k) d -> k c d",
                                                        k=P))
            bias_t = const.tile([P, D], F32, tag="bias")
            nc.sync.dma_start(bias_t[:], bias_ap[:])

            TGL = 4  # tiles per idx/dv load group (prefetch + fewer sync ops)
            grp_nb_max = max(int(NB_t[g:g + TGL].sum())
                             for g in range(0, nt, TGL))
            # scalar-block prefix offsets per group
            blk_of_t = np.concatenate([[0], np.cumsum(NB_t)]).astype(int)
            ns_before = np.concatenate([[0], np.cumsum(blk_scalar)]).astype(int)
            grp_ns_max = max(
                int(ns_before[blk_of_t[min(g + TGL, nt)]]
                    - ns_before[blk_of_t[g]]) for g in range(0, nt, TGL))
            grp_ns_max = max(grp_ns_max, 1)

            bo = 0  # global block offset
            idx_t = dvu_t = dvs_t = None
            gbo = 0   # block offset of current group start
            gso = 0   # scalar-block ordinal at current group start
            for t in range(nt):
                nb = int(NB_t[t])
                if t % TGL == 0:
                    gnb = int(NB_t[t:t + TGL].sum())
                    gbo = bo
                    idx_t = idxpool.tile([P, 8 * grp_nb_max], I16, tag="idx")
                    nc.sync.dma_start(idx_t[:, :8 * gnb],
                                      idxp_ap[:, 8 * bo:8 * (bo + gnb)])
                    dvu_t = dvupool.tile([P, 2 * grp_nb_max], F32, tag="dvu")
                    nc.sync.dma_start(dvu_t[:, :2 * gnb],
                                      dvu_ap[:, 2 * bo:2 * (bo + gnb)])
                    gso = int(ns_before[bo])
                    gns = int(ns_before[bo + gnb]) - gso
                    dvs_t = dvspool.tile([P, 3 * grp_ns_max], F32, tag="dvs")
                    if gns:
                        nc.sync.dma_start(dvs_t[:, :3 * gns],
                                          dvs_ap[:, 3 * gso:3 * (gso + gns)])
                lo = bo - gbo   # tile's block offset within the group tiles
                xg = xgpool.tile([P, nb_max, D], F16, tag="xg")
                ok = 0
                for k in range(NBANK):
                    nbk = int(nb_tk[t, k])
                    if nbk == 0:
                        continue
                    n = nbk * P
                    nc.gpsimd.dma_gather(
                        out_ap=xg[:, ok:ok + nbk, :],
                        in_ap=xb_aps[k][:],
                        idxs_ap=idx_t[:, 8 * (lo + ok):8 * (lo + ok + nbk)],
                        num_idxs=n,
                        num_idxs_reg=n,
                        elem_size=D,
                        # >64 descriptors (~1008 idxs) break the one-packet
                        # ceiling and wedge the exec unit
                        single_packet=(n <= 992),
                        queue_num=k,
                    )
                    ok += nbk

                z_ps = zpsum.tile([P, D], F32, tag="zps")
                for j in range(nb):
                    gj = bo + j
                    s_t = spool.tile([P, P], F16, tag="s")
                    if _is_scalar_block(gj):
                        # scalar engine: S = relu(val - val*|iota - dloc|)
                        c = 3 * (int(s_ord[gj]) - gso)
                        a_t = apool.tile([P, P], F16, tag="at")
                        nc.scalar.activation(
                            a_t[:], iota_t[:],
                            mybir.ActivationFunctionType.Abs,
                            bias=dvs_t[:, c:c + 1])
                        nc.scalar.activation(
                            s_t[:], a_t[:],
                            mybir.ActivationFunctionType.Relu,
                            scale=dvs_t[:, c + 1:c + 2],
                            bias=dvs_t[:, c + 2:c + 3])
                    else:
                        # integer ops on the f16 bit patterns (exact);
                        # DVE for most blocks, GpSimd for a slice of them
                        eng = (nc.gpsimd if _is_gpsimd_block(gj)
                               else nc.vector)
                        c = 2 * (lo + j)
                        eng.tensor_scalar(
                            out=s_t[:].bitcast(U16), in0=iotau_t[:],
                            scalar1=dvu_t[:, c:c + 1],
                            scalar2=dvu_t[:, c + 1:c + 2],
                            op0=mybir.AluOpType.is_equal,
                            op1=mybir.AluOpType.mult,
                        )
                    nc.tensor.matmul(out=z_ps[:], lhsT=s_t[:],
                                     rhs=xg[:, j, :],
                                     start=(j == 0), stop=(j == nb - 1))

                z_sb = epool.tile([P, D], F16, tag="zsb")
                nc.scalar.copy(z_sb[:], z_ps[:])
                o_ps = opsum.tile([P, D], F32, tag="ops")
                for ch in range(2):
                    zt_ps = tpsum.tile([P, P], F16, tag="ztps")
                    nc.tensor.transpose(zt_ps[:],
                                        z_sb[:, ch * P:(ch + 1) * P],
                                        ident_t[:])
                    zt_sb = epool.tile([P, P], F16, tag="ztsb")
                    nc.scalar.copy(zt_sb[:], zt_ps[:])
                    nc.tensor.matmul(out=o_ps[:], lhsT=zt_sb[:],
                                     rhs=w_t[:, ch, :],
                                     start=(ch == 0), stop=(ch == 1))
                o_sb = epool.tile([P, D], F32, tag="osb")
                nc.vector.tensor_add(o_sb[:], o_ps[:], bias_t[:])
                nc.sync.dma_start(out_ap[t * P:(t + 1) * P, :], o_sb[:])
                bo += nb
    nc.compile()
    return nc


def kernel(x, edge_row, edge_col, edge_val, weight, b):
    global _last_results
    assert x.shape == (N_NODES, D)

    nb_tk, scal_info, cores = _build_structure(
        np.asarray(edge_row), np.asarray(edge_col), np.asarray(edge_val))
    nc = _build_program(nb_tk, scal_info)

    x16 = np.asarray(x, np.float32).astype(np.float16)
    banks = [np.ascontiguousarray(x16[k * BS:(k + 1) * BS])
             for k in range(NBANK)]
    w16 = np.asarray(weight, np.float32).astype(np.float16)
    bias = np.broadcast_to(
        np.asarray(b, np.float32)[None, :], (P, D)).copy()
    iota = np.tile(np.arange(P, dtype=np.float16)[None, :], (P, 1))
    iotau = np.tile(np.arange(P, dtype=np.uint16)[None, :], (P, 1))
    ident = np.eye(P, dtype=np.float16)

    in_maps = []
    for c in range(NC):
        m = {f"xb{k}": banks[k] for k in range(NBANK)}
        m.update(idxp=cores[c]["idxp"], dvu=cores[c]["dvu"],
                 dvs=cores[c]["dvs"], w=w16, bias=bias, iota=iota,
                 iotau=iotau, ident=ident)
        in_maps.append(m)

    trace = bool(os.environ.get("KERNEL_TRACE"))
    res = run_bass_kernel_spmd(nc, in_maps, list(range(NC)), trace=trace)
    _last_results = res

    out = np.concatenate([res.results[c]["out"][:SH] for c in range(NC)],
                         axis=0)
    return out.astype(np.float32)


# revision 14
# speedup vs baseline: 1.3946x; 1.0011x over previous
"""GCNConv kernel for 8 Trainium2 NeuronCores (Bass/Tile).

Computes out = segment_sum(edge_val * (x @ W)[edge_col], edge_row) + b
as out = (A @ x) @ W + b  (associativity), with:
  - nodes (rows of output) sharded across 8 cores (12500 each)
  - edges partitioned by destination row -> per-core, per-128-row-tile
  - per 128-edge block: gather x[col] rows (fp16, 512B) via dma_gather,
    build a one-hot selection matrix S[e, dloc[e]] = val[e], and
    accumulate z[128 nodes, 256] += S.T @ X_block on the PE in PSUM.
  - S is built on the DVE as uint16 integer ops on the f16 BIT PATTERNS:
    T[e, j] = (j == dloc[e]) * f16bits(val[e]), bitcast to f16. Exact, and
    the 16-bit dtype unlocks the DVE 2x/4x acceleration modes. A fraction
    of blocks goes to the scalar engine (Abs+Relu) for balance.
  - epilogue per tile: transpose z, project by W (fp16), add bias, store.

x is split into 4 banks of 25000 rows because dma_gather indices are int16.
"""
import os
from contextlib import ExitStack

import numpy as np

import concourse.bass as bass
import concourse.tile as tile
from concourse import bacc, mybir
from concourse.bass_utils import run_bass_kernel_spmd

P = 128
D = 256
N_NODES = 100000
N_EDGES = 3200000
NC = 8
SH = N_NODES // NC          # 12500 rows per core
NT = (SH + P - 1) // P      # 98 tiles per core
NBANK = 4
BS = N_NODES // NBANK       # 25000 rows per bank (fits int16 index)

F16 = mybir.dt.float16
F32 = mybir.dt.float32
I16 = mybir.dt.int16
U16 = mybir.dt.uint16
U32 = mybir.dt.uint32

XG_BUFS = 6                 # gathered-x tile pool depth

# S-build engine assignment by global block index: ~22% of blocks go to
# the scalar engine (Abs+Relu), the rest to the DVE.
_last_results = None        # BassKernelResults of the most recent run


def _is_scalar_block(j):
    return j % 7 == 3


def _is_gpsimd_block(j):
    return False


def _build_structure(edge_row, edge_col, edge_val):
    """Sort/pad edges into per-core 128-edge blocks grouped by
    (dest tile, source bank).  Block structure (nb_tk) is shared across
    cores (padded to the max) so one SPMD program fits all cores.

    Returns (nb_tk [NT,NBANK] int, per-core dict arrays).
    """
    E = edge_row.shape[0]
    core = edge_row // SH
    r_loc = edge_row - core * SH
    t = r_loc // P
    dloc = (r_loc % P).astype(np.int64)
    bank = edge_col // BS
    bidx = (edge_col % BS).astype(np.int16)

    gid = (core.astype(np.int64) * NT + t) * NBANK + bank
    order = np.argsort(gid, kind="stable")
    gid_s = gid[order]

    cnt = np.bincount(gid, minlength=NC * NT * NBANK).reshape(NC, NT, NBANK)
    n_tk = cnt.max(axis=0)                           # exact max count [NT,NBANK]
    nb_tk = (n_tk + P - 1) // P                      # [NT, NBANK] blocks
    nb_tk = np.maximum(nb_tk, 1)                     # keep structure non-empty
    NB_t = nb_tk.sum(axis=1)                         # [NT]
    NBLK = int(NB_t.sum())
    pad_len = NBLK * P

    # slot offset of group (t,k) within a core's padded edge list
    off_tk = np.zeros((NT, NBANK), np.int64)
    flat_off = np.cumsum(nb_tk.ravel() * P)
    off_tk.ravel()[1:] = flat_off[:-1]

    # position of each edge within its (c,t,k) group
    grp_start = np.zeros(E, np.int64)
    newgrp = np.ones(E, bool)
    newgrp[1:] = gid_s[1:] != gid_s[:-1]
    starts = np.where(newgrp)[0]
    grp_start[starts] = starts
    grp_start = np.maximum.accumulate(grp_start)
    pos_in_grp = np.arange(E) - grp_start

    tk_of_edge = gid_s % (NT * NBANK)
    core_of_edge = gid_s // (NT * NBANK)
    dest = off_tk.ravel()[tk_of_edge] + pos_in_grp

    # scalar-block bookkeeping (shared across cores)
    blk_scalar = np.array([_is_scalar_block(j) for j in range(NBLK)])
    s_ord = np.cumsum(blk_scalar) - blk_scalar      # ordinal among scalar blks
    NSBLK = int(blk_scalar.sum())

    cores = []
    ev16 = edge_val.astype(np.float16)
    vbits = ev16.view(np.uint16)
    for c in range(NC):
        m = core_of_edge == c
        e_ids = order[m]
        d = dest[m]
        idx_arr = np.zeros(pad_len, np.int16)
        dloc_arr = np.zeros(pad_len, np.int64)
        vb_arr = np.zeros(pad_len, np.uint16)
        vf_arr = np.zeros(pad_len, np.float32)
        idx_arr[d] = bidx[e_ids]
        dloc_arr[d] = dloc[e_ids]
        vb_arr[d] = vbits[e_ids]
        vf_arr[d] = edge_val[e_ids].astype(np.float32)

        # packed gather indices: [128, 8*NBLK] int16 (16-wrap, replicated x8)
        idxp = np.tile(np.ascontiguousarray(idx_arr.reshape(-1, 16).T), (8, 1))

        # DVE path (uint32-packed S): per block scalars
        #   [dloc>>1, float(f16bits(val) << 16*(dloc&1))]  -> [128, 2*NBLK] f32
        # (bass requires f32 scalars; both ints have <=15 significant bits at
        # their magnitude so they are exact in f32)
        dl = np.ascontiguousarray(dloc_arr.reshape(NBLK, P).T)   # [128, NBLK]
        vb = np.ascontiguousarray(vb_arr.reshape(NBLK, P).T)
        vsh = vb.astype(np.uint32) << (16 * (dl & 1)).astype(np.uint32)
        dvu = np.empty((P, 2 * NBLK), np.float32)
        dvu[:, 0::2] = (dl >> 1).astype(np.float32)
        dvu[:, 1::2] = vsh.astype(np.float32)

        # scalar path: per scalar-block [-dloc, -val, +val] f32
        dvs = np.zeros((P, max(3 * NSBLK, 3)), np.float32)
        if NSBLK:
            dlf = dl.astype(np.float32)
            vf = np.ascontiguousarray(vf_arr.reshape(NBLK, P).T)
            sb = np.where(blk_scalar)[0]
            dvs[:, 0::3] = -dlf[:, sb]
            dvs[:, 1::3] = -vf[:, sb]
            dvs[:, 2::3] = vf[:, sb]
        cores.append(dict(idxp=idxp, dvu=dvu, dvs=dvs))

    return nb_tk, n_tk, (blk_scalar, s_ord, NSBLK), cores


def _build_program(nb_tk, n_tk, scal_info):
    """Build the SPMD Bass program for the given block structure."""
    nb_tk = np.asarray(nb_tk)
    n_tk = np.asarray(n_tk)
    blk_scalar, s_ord, NSBLK = scal_info
    NB_t = nb_tk.sum(axis=1)
    NBLK = int(NB_t.sum())
    nt = nb_tk.shape[0]
    out_rows = nt * P

    nc = bacc.Bacc("TRN2", target_bir_lowering=False, debug=False,
                   num_devices=NC, num_swdge_queues=4)
    xb_aps = [nc.dram_tensor(f"xb{k}", [BS, D], F16,
                             kind="ExternalInput").ap() for k in range(NBANK)]
    idxp_ap = nc.dram_tensor("idxp", [P, 8 * NBLK], I16,
                             kind="ExternalInput").ap()
    dvu_ap = nc.dram_tensor("dvu", [P, 2 * NBLK], F32,
                            kind="ExternalInput").ap()
    dvs_ap = nc.dram_tensor("dvs", [P, max(3 * NSBLK, 3)], F32,
                            kind="ExternalInput").ap()
    w_ap = nc.dram_tensor("w", [D, D], F16, kind="ExternalInput").ap()
    bias_ap = nc.dram_tensor("bias", [P, D], F32, kind="ExternalInput").ap()
    iota_ap = nc.dram_tensor("iota", [P, P], F16, kind="ExternalInput").ap()
    iotau_ap = nc.dram_tensor("iotau", [P, P // 2], U32,
                              kind="ExternalInput").ap()
    ident_ap = nc.dram_tensor("ident", [P, P], F16, kind="ExternalInput").ap()
    out_ap = nc.dram_tensor("out", [out_rows, D], F16,
                            kind="ExternalOutput").ap()

    nb_max = int(NB_t.max())

    with tile.TileContext(nc) as tc:
        with ExitStack() as ctx:
            const = ctx.enter_context(tc.tile_pool(name="const", bufs=1))
            idxpool = ctx.enter_context(tc.tile_pool(name="idxp", bufs=6))
            dvupool = ctx.enter_context(tc.tile_pool(name="dvup", bufs=6))
            dvspool = ctx.enter_context(tc.tile_pool(name="dvsp", bufs=6))
            xgpool = ctx.enter_context(tc.tile_pool(name="xgp", bufs=XG_BUFS))
            spool = ctx.enter_context(tc.tile_pool(name="sp", bufs=12))
            epool = ctx.enter_context(tc.tile_pool(name="ep", bufs=3))
            apool = ctx.enter_context(tc.tile_pool(name="ap", bufs=4))
            zpsum = ctx.enter_context(
                tc.tile_pool(name="zps", bufs=3, space="PSUM"))
            tpsum = ctx.enter_context(
                tc.tile_pool(name="tps", bufs=2, space="PSUM"))
            opsum = ctx.enter_context(
                tc.tile_pool(name="ops", bufs=2, space="PSUM"))

            iota_t = const.tile([P, P], F16, tag="iota")
            nc.sync.dma_start(iota_t[:], iota_ap[:])
            iotau_t = const.tile([P, P // 2], U32, tag="iotau")
            nc.sync.dma_start(iotau_t[:], iotau_ap[:])
            ident_t = const.tile([P, P], F16, tag="ident")
            nc.sync.dma_start(ident_t[:], ident_ap[:])
            w_t = const.tile([P, 2, D], F16, tag="w")
            nc.sync.dma_start(w_t[:], w_ap[:].rearrange("(c k) d -> k c d",
                                                        k=P))
            bias_t = const.tile([P, D], F32, tag="bias")
            nc.sync.dma_start(bias_t[:], bias_ap[:])

            TGL = 4  # tiles per idx/dv load group (prefetch + fewer sync ops)
            grp_nb_max = max(int(NB_t[g:g + TGL].sum())
                             for g in range(0, nt, TGL))
            # scalar-block prefix offsets per group
            blk_of_t = np.concatenate([[0], np.cumsum(NB_t)]).astype(int)
            ns_before = np.concatenate([[0], np.cumsum(blk_scalar)]).astype(int)
            grp_ns_max = max(
                int(ns_before[blk_of_t[min(g + TGL, nt)]]
                    - ns_before[blk_of_t[g]]) for g in range(0, nt, TGL))
            grp_ns_max = max(grp_ns_max, 1)

            bo = 0  # global block offset
            idx_t = dvu_t = dvs_t = None
            gbo = 0   # block offset of current group start
            gso = 0   # scalar-block ordinal at current group start
            for t in range(nt):
                nb = int(NB_t[t])
                if t % TGL == 0:
                    gnb = int(NB_t[t:t + TGL].sum())
                    gbo = bo
                    idx_t = idxpool.tile([P, 8 * grp_nb_max], I16, tag="idx")
                    nc.scalar.dma_start(idx_t[:, :8 * gnb],
                                        idxp_ap[:, 8 * bo:8 * (bo + gnb)])
                    dvu_t = dvupool.tile([P, 2 * grp_nb_max], F32, tag="dvu")
                    nc.scalar.dma_start(dvu_t[:, :2 * gnb],
                                        dvu_ap[:, 2 * bo:2 * (bo + gnb)])
                    gso = int(ns_before[bo])
                    gns = int(ns_before[bo + gnb]) - gso
                    dvs_t = dvspool.tile([P, 3 * grp_ns_max], F32, tag="dvs")
                    if gns:
                        nc.scalar.dma_start(dvs_t[:, :3 * gns],
                                            dvs_ap[:, 3 * gso:3 * (gso + gns)])
                lo = bo - gbo   # tile's block offset within the group tiles
                xg = xgpool.tile([P, nb_max, D], F16, tag="xg")
                ok = 0
                for k in range(NBANK):
                    nbk = int(nb_tk[t, k])
                    if nbk == 0:
                        continue
                    # exact count: skip gathering the block-padding slots.
                    # First XG_BUFS tiles gather the full padded range so
                    # every xg buffer byte is initialized (finite) before
                    # any stale-slot reuse.
                    n = nbk * P if t < XG_BUFS else max(int(n_tk[t, k]), 1)
                    nc.gpsimd.dma_gather(
                        out_ap=xg[:, ok:ok + nbk, :],
                        in_ap=xb_aps[k][:],
                        idxs_ap=idx_t[:, 8 * (lo + ok):8 * (lo + ok + nbk)],
                        num_idxs=n,
                        num_idxs_reg=n,
                        elem_size=D,
                        # >64 descriptors (~1008 idxs) break the one-packet
                        # ceiling and wedge the exec unit
                        single_packet=(n <= 992),
                        queue_num=k,
                    )
                    ok += nbk

                z_ps = zpsum.tile([P, D], F32, tag="zps")
                for j in range(nb):
                    gj = bo + j
                    s_t = spool.tile([P, P], F16, tag="s")
                    if _is_scalar_block(gj):
                        # scalar engine: S = relu(val - val*|iota - dloc|)
                        c = 3 * (int(s_ord[gj]) - gso)
                        a_t = apool.tile([P, P], F16, tag="at")
                        nc.scalar.activation(
                            a_t[:], iota_t[:],
                            mybir.ActivationFunctionType.Abs,
                            bias=dvs_t[:, c:c + 1])
                        nc.scalar.activation(
                            s_t[:], a_t[:],
                            mybir.ActivationFunctionType.Relu,
                            scale=dvs_t[:, c + 1:c + 2],
                            bias=dvs_t[:, c + 2:c + 3])
                    else:
                        # integer ops on the f16 bit patterns (exact);
                        # DVE for most blocks, GpSimd for a slice of them
                        eng = (nc.gpsimd if _is_gpsimd_block(gj)
                               else nc.vector)
                        c = 2 * (lo + j)
                        eng.tensor_scalar(
                            out=s_t[:].bitcast(U32), in0=iotau_t[:],
                            scalar1=dvu_t[:, c:c + 1],
                            scalar2=dvu_t[:, c + 1:c + 2],
                            op0=mybir.AluOpType.is_equal,
                            op1=mybir.AluOpType.mult,
                        )
                    nc.tensor.matmul(out=z_ps[:], lhsT=s_t[:],
                                     rhs=xg[:, j, :],
                                     start=(j == 0), stop=(j == nb - 1))

                z_sb = epool.tile([P, D], F16, tag="zsb")
                nc.scalar.copy(z_sb[:], z_ps[:])
                o_ps = opsum.tile([P, D], F32, tag="ops")
                for ch in range(2):
                    zt_ps = tpsum.tile([P, P], F16, tag="ztps")
                    nc.tensor.transpose(zt_ps[:],
                                        z_sb[:, ch * P:(ch + 1) * P],
                                        ident_t[:])
                    zt_sb = epool.tile([P, P], F16, tag="ztsb")
                    nc.scalar.copy(zt_sb[:], zt_ps[:])
                    nc.tensor.matmul(out=o_ps[:], lhsT=zt_sb[:],
                                     rhs=w_t[:, ch, :],
                                     start=(ch == 0), stop=(ch == 1))
                o_sb = epool.tile([P, D], F16, tag="osb")
                nc.vector.tensor_add(o_sb[:], o_ps[:], bias_t[:])
                nc.sync.dma_start(out_ap[t * P:(t + 1) * P, :], o_sb[:])
                bo += nb
    nc.compile()
    return nc


def kernel(x, edge_row, edge_col, edge_val, weight, b):
    global _last_results
    assert x.shape == (N_NODES, D)

    nb_tk, n_tk, scal_info, cores = _build_structure(
        np.asarray(edge_row), np.asarray(edge_col), np.asarray(edge_val))
    nc = _build_program(nb_tk, n_tk, scal_info)

    x16 = np.asarray(x, np.float32).astype(np.float16)
    banks = [np.ascontiguousarray(x16[k * BS:(k + 1) * BS])
             for k in range(NBANK)]
    w16 = np.asarray(weight, np.float32).astype(np.float16)
    bias = np.broadcast_to(
        np.asarray(b, np.float32)[None, :], (P, D)).copy()
    iota = np.tile(np.arange(P, dtype=np.float16)[None, :], (P, 1))
    iotau = np.tile(np.arange(P // 2, dtype=np.uint32)[None, :], (P, 1))
    ident = np.eye(P, dtype=np.float16)

    in_maps = []
    for c in range(NC):
        m = {f"xb{k}": banks[k] for k in range(NBANK)}
        m.update(idxp=cores[c]["idxp"], dvu=cores[c]["dvu"],
                 dvs=cores[c]["dvs"], w=w16, bias=bias, iota=iota,
                 iotau=iotau, ident=ident)
        in_maps.append(m)

    trace = bool(os.environ.get("KERNEL_TRACE"))
    res = run_bass_kernel_spmd(nc, in_maps, list(range(NC)), trace=trace)
    _last_results = res

    out = np.concatenate([res.results[c]["out"][:SH] for c in range(NC)],
                         axis=0)
    return out.astype(np.float32)
